# revision 8
# baseline (speedup 1.0000x reference)
"""CrossNet kernel for Trainium2 (Bass/Tile), data-parallel over 8 NeuronCores.

Reference computation (per layer l = 0..3):
    xw     = einsum('bd,d->b', x, w_l)
    x_next = x0 * xw[:, None] + b_l[None, :] + x

Algebraic restructure: every layer adds (x0 * scalar_per_row + const_row), so
    x_l = x0 * alpha_l[:, None] + y_l[None, :]
with
    t_l     = x0 @ w_l            (per-row scalars, all 4 from one thin matmul)
    c_l     = y_l . w_l           (host-computed layer constants)
    alpha_0 = 1,  alpha_{l+1} = alpha_l * (1 + t_l) + c_l
    y_0     = 0,  y_{l+1}     = y_l + b_l
The kernel computes t = x0 @ W^T on the PE (via on-chip PE transposes of x0
tiles), the tiny alpha recurrence on the DVE, and the final scale
out = x0 * alpha_L (+ y_L) as one elementwise pass. This is numerically
equivalent to the reference up to fp32 rounding (~3e-7 rel err) and makes the
problem HBM-bandwidth-bound (one read + one write of the activation tensor).

Sharding: batch dim split across 8 cores (2048 rows each); weights replicated.
"""

import numpy as np

import concourse.bass as bass
import concourse.mybir as mybir
import concourse.tile as tile
from concourse import bacc
from concourse.bass_utils import run_bass_kernel_spmd
from concourse.masks import make_identity

N_CORES = 8
B, D, L = 16384, 1024, 4
B_LOC = B // N_CORES  # 2048 rows per core
P = 128               # SBUF partitions
N_TILES = B_LOC // P  # 16 batch tiles per core
N_DCH = D // P        # 8 contraction chunks of 128

F32 = mybir.dt.float32


def _build_program(zero_bias: bool, c_consts, reps: int = 1):
    """Emit the per-core Bass program (SPMD: same NEFF on all 8 cores).

    reps > 1 repeats the whole computation back-to-back inside one NEFF —
    used only for timing (per-iteration slope cancels dispatch overhead).
    """
    nc = bacc.Bacc("TRN2", target_bir_lowering=False, debug=False)

    x_dram = nc.dram_tensor("x", [B_LOC, D], F32, kind="ExternalInput")
    wT_dram = nc.dram_tensor("wT", [P, N_DCH, L], F32, kind="ExternalInput")
    if not zero_bias:
        yl_dram = nc.dram_tensor("yL", [1, D], F32, kind="ExternalInput")
    out_dram = nc.dram_tensor("out", [B_LOC, D], F32, kind="ExternalOutput")

    with tile.TileContext(nc) as tc:
        with (
            tc.tile_pool(name="consts", bufs=1) as consts,
            tc.tile_pool(name="xp", bufs=6) as xp,
            tc.tile_pool(name="xtp", bufs=4) as xtp,
            tc.tile_pool(name="outp", bufs=6) as outp,
            tc.tile_pool(name="small", bufs=12) as small,
            tc.tile_pool(name="ptr", bufs=6, space="PSUM") as ptr,
            tc.tile_pool(name="ptt", bufs=2, space="PSUM") as ptt,
        ):
            identity = consts.tile([P, P], F32)
            make_identity(nc, identity)

            wT = consts.tile([P, N_DCH, L], F32)
            nc.sync.dma_start(out=wT, in_=wT_dram[:])

            if not zero_bias:
                # y_L row, replicated across all 128 partitions.
                ylb = consts.tile([P, D], F32)
                yl_ap = yl_dram[:]
                yl_bcast = bass.AP(
                    tensor=yl_ap.tensor,
                    offset=yl_ap.offset,
                    ap=[[0, P], yl_ap.ap[1]],
                )
                nc.sync.dma_start(out=ylb, in_=yl_bcast)

            for _rep in range(reps):
              for i in range(N_TILES):
                x_t = xp.tile([P, D], F32)
                nc.sync.dma_start(out=x_t, in_=x_dram[i * P : (i + 1) * P, :])

                # Transpose the 8 [128,128] blocks of x_t on the PE so the
                # contraction dim (d) lands on partitions; stage via PSUM.
                xT = xtp.tile([P, D], F32)
                for g in range(2):
                    pt_tr = ptr.tile([P, 512], F32)
                    for jj in range(4):
                        j = g * 4 + jj
                        nc.tensor.transpose(
                            pt_tr[:, jj * P : (jj + 1) * P],
                            x_t[:, j * P : (j + 1) * P],
                            identity,
                        )
                    # PSUM->SBUF staging copies on ACT (keeps DVE free for
                    # the alpha recurrence + final scale).
                    nc.scalar.copy(xT[:, g * 512 : (g + 1) * 512], pt_tr[:])

                # t tile [128 rows, 4 layers] accumulated over 8 d-chunks.
                t_ps = ptt.tile([P, L], F32)
                for j in range(N_DCH):
                    nc.tensor.matmul(
                        t_ps[:],
                        xT[:, j * P : (j + 1) * P],
                        wT[:, j, :],
                        start=(j == 0),
                        stop=(j == N_DCH - 1),
                    )

                # alpha = prod_l (1 + t_l)  (+ c_l terms when biases != 0)
                alpha = small.tile([P, 1], F32)
                if zero_bias:
                    tp1 = small.tile([P, L], F32)
                    nc.vector.tensor_scalar_add(tp1, t_ps[:], 1.0)
                    tm = small.tile([P, 2], F32)
                    nc.vector.tensor_tensor(
                        tm, tp1[:, 0:2], tp1[:, 2:4], mybir.AluOpType.mult
                    )
                    nc.vector.tensor_tensor(
                        alpha, tm[:, 0:1], tm[:, 1:2], mybir.AluOpType.mult
                    )
                else:
                    a_cur = None
                    for l in range(L):
                        anew = small.tile([P, 1], F32, name=f"a{l}_{i}")
                        if a_cur is None:
                            nc.vector.tensor_scalar_add(
                                anew, t_ps[:, l : l + 1], 1.0 + float(c_consts[l])
                            )
                        else:
                            nc.vector.tensor_scalar(
                                anew,
                                t_ps[:, l : l + 1],
                                1.0,
                                a_cur[:],
                                mybir.AluOpType.add,
                                mybir.AluOpType.mult,
                            )
                            if float(c_consts[l]) != 0.0:
                                nc.vector.tensor_scalar_add(
                                    anew, anew[:], float(c_consts[l])
                                )
                        a_cur = anew
                    nc.vector.tensor_copy(alpha, a_cur[:])

                # out = x0 * alpha (+ y_L)
                o_t = outp.tile([P, D], F32)
                if zero_bias:
                    nc.vector.tensor_scalar_mul(o_t, x_t[:], alpha[:])
                else:
                    nc.vector.scalar_tensor_tensor(
                        o_t,
                        x_t[:],
                        alpha[:],
                        ylb[:],
                        mybir.AluOpType.mult,
                        mybir.AluOpType.add,
                    )
                # Output DMAs triggered from gpsimd so their dispatch doesn't
                # serialize behind the input DMAs on the sync sequencer.
                nc.gpsimd.dma_start(out=out_dram[i * P : (i + 1) * P, :], in_=o_t)

    nc.compile()
    return nc


_CACHE = {}


def _get_program(zero_bias: bool, c_key):
    key = (zero_bias, c_key)
    if key not in _CACHE:
        c_consts = list(c_key) if c_key is not None else None
        _CACHE[key] = _build_program(zero_bias, c_consts)
    return _CACHE[key]


def kernel(inputs, weights, biases, _trace=False, _bass_results=None):
    inputs = np.ascontiguousarray(np.asarray(inputs, dtype=np.float32))
    weights = np.ascontiguousarray(np.asarray(weights, dtype=np.float32))
    biases = np.ascontiguousarray(np.asarray(biases, dtype=np.float32))
    assert inputs.shape == (B, D) and weights.shape == (L, D) and biases.shape == (L, D)

    zero_bias = bool(np.all(biases == 0.0))

    # Host-side prep of the tiny replicated weight tensors.
    # wT[p, j, l] = W[l, j*128 + p]  (W^T in d-chunked, partition-major layout)
    wT_np = np.ascontiguousarray(
        weights.T.reshape(N_DCH, P, L).transpose(1, 0, 2)
    ).astype(np.float32)

    if zero_bias:
        c_key = None
        yl_np = None
    else:
        # y_l = sum_{j<l} b_j ;  c_l = y_l . w_l
        y = np.zeros(D, dtype=np.float64)
        c = []
        for l in range(L):
            c.append(float(np.dot(y, weights[l].astype(np.float64))))
            y = y + biases[l].astype(np.float64)
        c_key = tuple(c)
        yl_np = np.ascontiguousarray(y.astype(np.float32).reshape(1, D))

    nc = _get_program(zero_bias, c_key)

    in_maps = []
    for core in range(N_CORES):
        m = {
            "x": inputs[core * B_LOC : (core + 1) * B_LOC],
            "wT": wT_np,
        }
        if not zero_bias:
            m["yL"] = yl_np
        in_maps.append(m)

    res = run_bass_kernel_spmd(
        nc, in_maps, core_ids=list(range(N_CORES)), trace=_trace
    )
    if _bass_results is not None:
        _bass_results.append(res)

    out = np.concatenate([res.results[c]["out"] for c in range(N_CORES)], axis=0)
    return out


# revision 11
# speedup vs baseline: 1.8793x; 1.8793x over previous
"""CrossNet kernel for Trainium2 (Bass/Tile), data-parallel over 8 NeuronCores.

Reference computation (per layer l = 0..3):
    xw     = einsum('bd,d->b', x, w_l)
    x_next = x0 * xw[:, None] + b_l[None, :] + x

Algebraic restructure: every layer adds (x0 * scalar_per_row + const_row), so
    x_l = x0 * alpha_l[:, None] + y_l[None, :]
with
    t_l     = x0 @ w_l            (per-row scalars, all 4 from one thin matmul)
    c_l     = y_l . w_l           (host-computed layer constants)
    alpha_0 = 1,  alpha_{l+1} = alpha_l * (1 + t_l) + c_l
    y_0     = 0,  y_{l+1}     = y_l + b_l
The kernel computes t = x0 @ W^T on the PE (via on-chip PE transposes of x0
tiles), the tiny alpha recurrence on the DVE, and the final scale
out = x0 * alpha_L (+ y_L) as one elementwise pass. This is numerically
equivalent to the reference up to fp32 rounding (~3e-7 rel err) and makes the
problem HBM-bandwidth-bound (one read + one write of the activation tensor).

Sharding: batch dim split across 8 cores (2048 rows each); weights replicated.
"""

import numpy as np

import concourse.bass as bass
import concourse.mybir as mybir
import concourse.tile as tile
from concourse import bacc
from concourse.bass_utils import run_bass_kernel_spmd
from concourse.masks import make_identity

N_CORES = 8
B, D, L = 16384, 1024, 4
B_LOC = B // N_CORES  # 2048 rows per core
P = 128               # SBUF partitions
N_TILES = B_LOC // P  # 16 batch tiles per core
N_DCH = D // P        # 8 contraction chunks of 128

F32 = mybir.dt.float32


def _build_program(zero_bias: bool, c_consts, reps: int = 1, hw_loop: int = 0):
    """Emit the per-core Bass program (SPMD: same NEFF on all 8 cores).

    reps > 1 repeats the whole computation back-to-back inside one NEFF;
    hw_loop = K > 0 wraps the body in a hardware For_i loop running K times.
    Both are used only for timing (slopes cancel dispatch overhead).
    """
    nc = bacc.Bacc("TRN2", target_bir_lowering=False, debug=False)

    x_dram = nc.dram_tensor("x", [B_LOC, D], F32, kind="ExternalInput")
    wT_dram = nc.dram_tensor("wT", [P, N_DCH, L], F32, kind="ExternalInput")
    if not zero_bias:
        yl_dram = nc.dram_tensor("yL", [1, D], F32, kind="ExternalInput")
    out_dram = nc.dram_tensor("out", [B_LOC, D], F32, kind="ExternalOutput")

    with tile.TileContext(nc) as tc:
        with (
            tc.tile_pool(name="consts", bufs=1) as consts,
            tc.tile_pool(name="xp", bufs=6) as xp,
            tc.tile_pool(name="xtp", bufs=4) as xtp,
            tc.tile_pool(name="outp", bufs=6) as outp,
            tc.tile_pool(name="small", bufs=12) as small,
            tc.tile_pool(name="ptr", bufs=6, space="PSUM") as ptr,
            tc.tile_pool(name="ptt", bufs=2, space="PSUM") as ptt,
        ):
            identity = consts.tile([P, P], F32)
            make_identity(nc, identity)

            wT = consts.tile([P, N_DCH, L], F32)
            nc.sync.dma_start(out=wT, in_=wT_dram[:])

            if not zero_bias:
                # y_L row, replicated across all 128 partitions.
                ylb = consts.tile([P, D], F32)
                yl_ap = yl_dram[:]
                yl_bcast = bass.AP(
                    tensor=yl_ap.tensor,
                    offset=yl_ap.offset,
                    ap=[[0, P], yl_ap.ap[1]],
                )
                nc.sync.dma_start(out=ylb, in_=yl_bcast)

            def emit_tile(i):
                x_t = xp.tile([P, D], F32)
                nc.sync.dma_start(out=x_t, in_=x_dram[i * P : (i + 1) * P, :])

                # Transpose the 8 [128,128] blocks of x_t on the PE so the
                # contraction dim (d) lands on partitions; stage via PSUM.
                xT = xtp.tile([P, D], F32)
                for g in range(2):
                    pt_tr = ptr.tile([P, 512], F32)
                    for jj in range(4):
                        j = g * 4 + jj
                        nc.tensor.transpose(
                            pt_tr[:, jj * P : (jj + 1) * P],
                            x_t[:, j * P : (j + 1) * P],
                            identity,
                        )
                    # PSUM->SBUF staging copies on ACT (keeps DVE free for
                    # the alpha recurrence + final scale).
                    nc.scalar.copy(xT[:, g * 512 : (g + 1) * 512], pt_tr[:])

                # t tile [128 rows, 4 layers] accumulated over 8 d-chunks.
                t_ps = ptt.tile([P, L], F32)
                for j in range(N_DCH):
                    nc.tensor.matmul(
                        t_ps[:],
                        xT[:, j * P : (j + 1) * P],
                        wT[:, j, :],
                        start=(j == 0),
                        stop=(j == N_DCH - 1),
                    )

                # alpha = prod_l (1 + t_l)  (+ c_l terms when biases != 0)
                alpha = small.tile([P, 1], F32)
                if zero_bias:
                    tp1 = small.tile([P, L], F32)
                    nc.vector.tensor_scalar_add(tp1, t_ps[:], 1.0)
                    tm = small.tile([P, 2], F32)
                    nc.vector.tensor_tensor(
                        tm, tp1[:, 0:2], tp1[:, 2:4], mybir.AluOpType.mult
                    )
                    nc.vector.tensor_tensor(
                        alpha, tm[:, 0:1], tm[:, 1:2], mybir.AluOpType.mult
                    )
                else:
                    a_cur = None
                    for l in range(L):
                        anew = small.tile([P, 1], F32, name=f"a{l}_{i}")
                        if a_cur is None:
                            nc.vector.tensor_scalar_add(
                                anew, t_ps[:, l : l + 1], 1.0 + float(c_consts[l])
                            )
                        else:
                            nc.vector.tensor_scalar(
                                anew,
                                t_ps[:, l : l + 1],
                                1.0,
                                a_cur[:],
                                mybir.AluOpType.add,
                                mybir.AluOpType.mult,
                            )
                            if float(c_consts[l]) != 0.0:
                                nc.vector.tensor_scalar_add(
                                    anew, anew[:], float(c_consts[l])
                                )
                        a_cur = anew
                    nc.vector.tensor_copy(alpha, a_cur[:])

                # out = x0 * alpha (+ y_L)
                o_t = outp.tile([P, D], F32)
                if zero_bias:
                    nc.vector.tensor_scalar_mul(o_t, x_t[:], alpha[:])
                else:
                    nc.vector.scalar_tensor_tensor(
                        o_t,
                        x_t[:],
                        alpha[:],
                        ylb[:],
                        mybir.AluOpType.mult,
                        mybir.AluOpType.add,
                    )
                # Output DMAs triggered from gpsimd so their dispatch doesn't
                # serialize behind the input DMAs on the sync sequencer.
                nc.gpsimd.dma_start(out=out_dram[i * P : (i + 1) * P, :], in_=o_t)

            if hw_loop > 0:
                with tc.For_i(
                    0, hw_loop, 1, hint_engines=(mybir.EngineType.PE,)
                ) as _iv:
                    for i in range(N_TILES):
                        emit_tile(i)
            else:
                for _rep in range(reps):
                    for i in range(N_TILES):
                        emit_tile(i)

    nc.compile()
    return nc


_CACHE = {}


def _get_program(zero_bias: bool, c_key):
    key = (zero_bias, c_key)
    if key not in _CACHE:
        c_consts = list(c_key) if c_key is not None else None
        _CACHE[key] = _build_program(zero_bias, c_consts)
    return _CACHE[key]


def kernel(inputs, weights, biases, _trace=False, _bass_results=None):
    inputs = np.ascontiguousarray(np.asarray(inputs, dtype=np.float32))
    weights = np.ascontiguousarray(np.asarray(weights, dtype=np.float32))
    biases = np.ascontiguousarray(np.asarray(biases, dtype=np.float32))
    assert inputs.shape == (B, D) and weights.shape == (L, D) and biases.shape == (L, D)

    zero_bias = bool(np.all(biases == 0.0))

    # Host-side prep of the tiny replicated weight tensors.
    # wT[p, j, l] = W[l, j*128 + p]  (W^T in d-chunked, partition-major layout)
    wT_np = np.ascontiguousarray(
        weights.T.reshape(N_DCH, P, L).transpose(1, 0, 2)
    ).astype(np.float32)

    if zero_bias:
        c_key = None
        yl_np = None
    else:
        # y_l = sum_{j<l} b_j ;  c_l = y_l . w_l
        y = np.zeros(D, dtype=np.float64)
        c = []
        for l in range(L):
            c.append(float(np.dot(y, weights[l].astype(np.float64))))
            y = y + biases[l].astype(np.float64)
        c_key = tuple(c)
        yl_np = np.ascontiguousarray(y.astype(np.float32).reshape(1, D))

    nc = _get_program(zero_bias, c_key)

    in_maps = []
    for core in range(N_CORES):
        m = {
            "x": inputs[core * B_LOC : (core + 1) * B_LOC],
            "wT": wT_np,
        }
        if not zero_bias:
            m["yL"] = yl_np
        in_maps.append(m)

    res = run_bass_kernel_spmd(
        nc, in_maps, core_ids=list(range(N_CORES)), trace=_trace
    )
    if _bass_results is not None:
        _bass_results.append(res)

    out = np.concatenate([res.results[c]["out"] for c in range(N_CORES)], axis=0)
    return out
